# revision 2
# baseline (speedup 1.0000x reference)
"""GAE (Generalized Advantage Estimation) Bass kernel for 8 Trainium2 cores.

Problem: rewards (2048, 8192) f32, values (2048, 8192) f32,
next_values (2048,) f32.
  next_v[:, t] = values[:, t+1] (t < S-1), next_values (t = S-1)
  deltas = rewards + GAMMA * next_v - values
  A_t = deltas_t + (GAMMA*LAM) * A_{t+1}   (A_S = 0, backward recurrence)
  advantages = A, returns = A + values

Sharding: pure data parallel over the batch dim — 2048 rows / 8 cores =
256 rows per core; the seq recurrence is row-local so there is no
cross-core communication.

Per core: 2 partition tiles of 128 rows x 8192 seq. The seq dim is cut
into chunks; each chunk's backward recurrence runs as a single DVE
tensor_tensor_scan over a reversed (negative-stride) view, chained
right-to-left across chunks through the scan's `initial` operand.
Everything stays in the natural [batch, seq] layout, so all DMAs move
long contiguous runs and the kernel is HBM-bound (~32MB/core traffic).
"""

import sys

if "/opt/trn_rl_repo" not in sys.path:
    sys.path.insert(0, "/opt/trn_rl_repo")

import numpy as np

GAMMA = 0.99
LAM = 0.95
C_COEF = GAMMA * LAM

B, S = 2048, 8192
N_CORES = 8
ROWS = B // N_CORES  # 256 rows per core
P = 128  # SBUF partitions
N_TILES = ROWS // P  # 2 row-tiles per core
CHUNK = 2048
N_CHUNKS = S // CHUNK

_CACHE: dict = {}


def _build():
    import concourse.bacc as bacc
    import concourse.mybir as mybir
    from concourse.tile import TileContext

    f32 = mybir.dt.float32
    add = mybir.AluOpType.add
    sub = mybir.AluOpType.subtract
    mult = mybir.AluOpType.mult

    nc = bacc.Bacc("TRN2", target_bir_lowering=False, name="gae8")
    r = nc.dram_tensor("rewards", [ROWS, S], f32, kind="ExternalInput")
    v = nc.dram_tensor("values", [ROWS, S], f32, kind="ExternalInput")
    nv = nc.dram_tensor("next_values", [ROWS], f32, kind="ExternalInput")
    adv = nc.dram_tensor("adv", [ROWS, S], f32, kind="ExternalOutput")
    ret = nc.dram_tensor("ret", [ROWS, S], f32, kind="ExternalOutput")

    # Returns satisfy their own backward recurrence, which needs one fewer
    # elementwise pass than the advantages form:
    #   B_t = e_t + c*B_{t+1},  e_t = r_t + gamma*(1-lam)*v_{t+1},  B_S = nv
    #   returns = B, advantages = B - v
    g1ml = GAMMA * (1.0 - LAM)

    with TileContext(nc) as tc:
        with (
            tc.tile_pool(name="cpool", bufs=1) as cpool,
            tc.tile_pool(name="pool", bufs=4) as pool,
        ):
            c_t = cpool.tile([P, 1], f32)
            nc.vector.memset(c_t[:, :], C_COEF)

            for t in range(N_TILES):
                rows = slice(t * P, (t + 1) * P)
                prev_ret = None
                # right-to-left so the scan carry chains across chunks
                for k in range(N_CHUNKS - 1, -1, -1):
                    col = k * CHUNK
                    v_t = pool.tile([P, CHUNK + 1], f32)
                    r_t = pool.tile([P, CHUNK], f32)
                    ret_t = pool.tile([P, CHUNK], f32)
                    adv_t = pool.tile([P, CHUNK], f32)

                    # v_t holds CHUNK+1 columns: v[:, col:col+CHUNK] plus the
                    # successor column (next chunk's first value, or
                    # next_values at the right edge).
                    if k == N_CHUNKS - 1:
                        nc.sync.dma_start(
                            out=v_t[:, 0:CHUNK], in_=v[rows, col : col + CHUNK]
                        )
                        nc.sync.dma_start(
                            out=v_t[:, CHUNK : CHUNK + 1],
                            in_=nv[t * P : (t + 1) * P].unsqueeze(1),
                        )
                    else:
                        nc.sync.dma_start(
                            out=v_t[:, :], in_=v[rows, col : col + CHUNK + 1]
                        )
                    nc.sync.dma_start(out=r_t[:, :], in_=r[rows, col : col + CHUNK])

                    # e = g1ml * v_next + r  (in place over r_t)
                    nc.vector.scalar_tensor_tensor(
                        out=r_t[:, :],
                        in0=v_t[:, 1 : CHUNK + 1],
                        scalar=g1ml,
                        in1=r_t[:, :],
                        op0=mult,
                        op1=add,
                    )
                    # backward recurrence over reversed views:
                    # state = c*state + e -> returns; carry chains via initial.
                    # Rightmost chunk: initial = next_values (v_t's extra col).
                    init = (
                        v_t[:, CHUNK : CHUNK + 1]
                        if prev_ret is None
                        else prev_ret[:, 0:1]
                    )
                    nc.vector.tensor_tensor_scan(
                        out=ret_t[:, ::-1],
                        data0=c_t[:, :].broadcast_to([P, CHUNK]),
                        data1=r_t[:, ::-1],
                        initial=init,
                        op0=mult,
                        op1=add,
                    )
                    # advantages = returns - v (GpSimd: keeps DVE free for scans)
                    nc.gpsimd.tensor_tensor(
                        out=adv_t[:, :],
                        in0=ret_t[:, :],
                        in1=v_t[:, 0:CHUNK],
                        op=sub,
                    )

                    nc.sync.dma_start(
                        out=ret[rows, col : col + CHUNK], in_=ret_t[:, :]
                    )
                    nc.sync.dma_start(
                        out=adv[rows, col : col + CHUNK], in_=adv_t[:, :]
                    )
                    prev_ret = ret_t
    nc.finalize()
    return nc


def _get_nc():
    if "nc" not in _CACHE:
        _CACHE["nc"] = _build()
    return _CACHE["nc"]


def _run(rewards, values, next_values, **spmd_kwargs):
    """Shard over cores, run the Bass kernel, return BassKernelResults."""
    from concourse.bass_utils import run_bass_kernel_spmd

    nc = _get_nc()
    in_maps = []
    for c in range(N_CORES):
        sl = slice(c * ROWS, (c + 1) * ROWS)
        in_maps.append(
            {
                "rewards": np.ascontiguousarray(rewards[sl], dtype=np.float32),
                "values": np.ascontiguousarray(values[sl], dtype=np.float32),
                "next_values": np.ascontiguousarray(
                    next_values[sl], dtype=np.float32
                ),
            }
        )
    return run_bass_kernel_spmd(
        nc, in_maps, core_ids=list(range(N_CORES)), **spmd_kwargs
    )


def kernel(rewards, values, next_values):
    res = _run(rewards, values, next_values)
    advantages = np.concatenate([res.results[c]["adv"] for c in range(N_CORES)], 0)
    returns = np.concatenate([res.results[c]["ret"] for c in range(N_CORES)], 0)
    return advantages, returns


# revision 3
# speedup vs baseline: 1.1426x; 1.1426x over previous
"""GAE (Generalized Advantage Estimation) Bass kernel for 8 Trainium2 cores.

Problem: rewards (2048, 8192) f32, values (2048, 8192) f32,
next_values (2048,) f32.
  next_v[:, t] = values[:, t+1] (t < S-1), next_values (t = S-1)
  deltas = rewards + GAMMA * next_v - values
  A_t = deltas_t + (GAMMA*LAM) * A_{t+1}   (A_S = 0, backward recurrence)
  advantages = A, returns = A + values

Sharding: pure data parallel over the batch dim — 2048 rows / 8 cores =
256 rows per core; the seq recurrence is row-local so there is no
cross-core communication.

Per core: 2 partition tiles of 128 rows x 8192 seq. The seq dim is cut
into chunks; each chunk's backward recurrence runs as a single DVE
tensor_tensor_scan over a reversed (negative-stride) view, chained
right-to-left across chunks through the scan's `initial` operand.
Everything stays in the natural [batch, seq] layout, so all DMAs move
long contiguous runs and the kernel is HBM-bound (~32MB/core traffic).
"""

import sys

if "/opt/trn_rl_repo" not in sys.path:
    sys.path.insert(0, "/opt/trn_rl_repo")

import numpy as np

GAMMA = 0.99
LAM = 0.95
C_COEF = GAMMA * LAM

B, S = 2048, 8192
N_CORES = 8
ROWS = B // N_CORES  # 256 rows per core
P = 128  # SBUF partitions
N_TILES = ROWS // P  # 2 row-tiles per core
CHUNK = 2048
N_CHUNKS = S // CHUNK

_CACHE: dict = {}


def _build():
    import concourse.bacc as bacc
    import concourse.mybir as mybir
    from concourse.tile import TileContext

    f32 = mybir.dt.float32
    add = mybir.AluOpType.add
    sub = mybir.AluOpType.subtract
    mult = mybir.AluOpType.mult

    nc = bacc.Bacc("TRN2", target_bir_lowering=False, name="gae8")
    r = nc.dram_tensor("rewards", [ROWS, S], f32, kind="ExternalInput")
    v = nc.dram_tensor("values", [ROWS, S], f32, kind="ExternalInput")
    nv = nc.dram_tensor("next_values", [ROWS], f32, kind="ExternalInput")
    adv = nc.dram_tensor("adv", [ROWS, S], f32, kind="ExternalOutput")
    ret = nc.dram_tensor("ret", [ROWS, S], f32, kind="ExternalOutput")

    # Returns satisfy their own backward recurrence, which needs one fewer
    # elementwise pass than the advantages form:
    #   B_t = e_t + c*B_{t+1},  e_t = r_t + gamma*(1-lam)*v_{t+1},  B_S = nv
    #   returns = B, advantages = B - v
    g1ml = GAMMA * (1.0 - LAM)

    with TileContext(nc) as tc:
        with (
            tc.tile_pool(name="cpool", bufs=1) as cpool,
            tc.tile_pool(name="pool", bufs=4) as pool,
        ):
            c_t = cpool.tile([P, 1], f32)
            nc.vector.memset(c_t[:, :], C_COEF)

            for t in range(N_TILES):
                rows = slice(t * P, (t + 1) * P)
                prev_ret = None
                # right-to-left so the scan carry chains across chunks
                for k in range(N_CHUNKS - 1, -1, -1):
                    col = k * CHUNK
                    v_t = pool.tile([P, CHUNK + 1], f32)
                    r_t = pool.tile([P, CHUNK], f32)
                    ret_t = pool.tile([P, CHUNK], f32)
                    adv_t = pool.tile([P, CHUNK], f32)

                    # v_t holds CHUNK+1 columns: v[:, col:col+CHUNK] plus the
                    # successor column (next chunk's first value, or
                    # next_values at the right edge).
                    if k == N_CHUNKS - 1:
                        nc.sync.dma_start(
                            out=v_t[:, 0:CHUNK], in_=v[rows, col : col + CHUNK]
                        )
                        nc.sync.dma_start(
                            out=v_t[:, CHUNK : CHUNK + 1],
                            in_=nv[t * P : (t + 1) * P].unsqueeze(1),
                        )
                    else:
                        nc.sync.dma_start(
                            out=v_t[:, :], in_=v[rows, col : col + CHUNK + 1]
                        )
                    nc.sync.dma_start(out=r_t[:, :], in_=r[rows, col : col + CHUNK])

                    # e = g1ml * v_next + r  (in place over r_t)
                    nc.vector.scalar_tensor_tensor(
                        out=r_t[:, :],
                        in0=v_t[:, 1 : CHUNK + 1],
                        scalar=g1ml,
                        in1=r_t[:, :],
                        op0=mult,
                        op1=add,
                    )
                    # backward recurrence over reversed views:
                    # state = c*state + e -> returns; carry chains via initial.
                    # Rightmost chunk: initial = next_values (v_t's extra col).
                    init = (
                        v_t[:, CHUNK : CHUNK + 1]
                        if prev_ret is None
                        else prev_ret[:, 0:1]
                    )
                    nc.vector.tensor_tensor_scan(
                        out=ret_t[:, ::-1],
                        data0=c_t[:, :].broadcast_to([P, CHUNK]),
                        data1=r_t[:, ::-1],
                        initial=init,
                        op0=mult,
                        op1=add,
                    )
                    # advantages = returns - v (DVE: GpSimd contends for SBUF
                    # ports with DVE and halves scan throughput)
                    nc.vector.tensor_tensor(
                        out=adv_t[:, :],
                        in0=ret_t[:, :],
                        in1=v_t[:, 0:CHUNK],
                        op=sub,
                    )

                    nc.sync.dma_start(
                        out=ret[rows, col : col + CHUNK], in_=ret_t[:, :]
                    )
                    nc.sync.dma_start(
                        out=adv[rows, col : col + CHUNK], in_=adv_t[:, :]
                    )
                    prev_ret = ret_t
    nc.finalize()
    return nc


def _get_nc():
    if "nc" not in _CACHE:
        _CACHE["nc"] = _build()
    return _CACHE["nc"]


def _run(rewards, values, next_values, **spmd_kwargs):
    """Shard over cores, run the Bass kernel, return BassKernelResults."""
    from concourse.bass_utils import run_bass_kernel_spmd

    nc = _get_nc()
    in_maps = []
    for c in range(N_CORES):
        sl = slice(c * ROWS, (c + 1) * ROWS)
        in_maps.append(
            {
                "rewards": np.ascontiguousarray(rewards[sl], dtype=np.float32),
                "values": np.ascontiguousarray(values[sl], dtype=np.float32),
                "next_values": np.ascontiguousarray(
                    next_values[sl], dtype=np.float32
                ),
            }
        )
    return run_bass_kernel_spmd(
        nc, in_maps, core_ids=list(range(N_CORES)), **spmd_kwargs
    )


def kernel(rewards, values, next_values):
    res = _run(rewards, values, next_values)
    advantages = np.concatenate([res.results[c]["adv"] for c in range(N_CORES)], 0)
    returns = np.concatenate([res.results[c]["ret"] for c in range(N_CORES)], 0)
    return advantages, returns


# revision 4
# speedup vs baseline: 1.5069x; 1.3188x over previous
"""GAE (Generalized Advantage Estimation) Bass kernel for 8 Trainium2 cores.

Problem: rewards (2048, 8192) f32, values (2048, 8192) f32,
next_values (2048,) f32.
  next_v[:, t] = values[:, t+1] (t < S-1), next_values (t = S-1)
  deltas = rewards + GAMMA * next_v - values
  A_t = deltas_t + (GAMMA*LAM) * A_{t+1}   (A_S = 0, backward recurrence)
  advantages = A, returns = A + values

Sharding: pure data parallel over the batch dim — 2048 rows / 8 cores =
256 rows per core; the seq recurrence is row-local so there is no
cross-core communication.

Per core: 2 partition tiles of 128 rows x 8192 seq. The seq dim is cut
into chunks; each chunk's backward recurrence runs as a single DVE
tensor_tensor_scan over a reversed (negative-stride) view, chained
right-to-left across chunks through the scan's `initial` operand.
Everything stays in the natural [batch, seq] layout, so all DMAs move
long contiguous runs and the kernel is HBM-bound (~32MB/core traffic).
"""

import sys

if "/opt/trn_rl_repo" not in sys.path:
    sys.path.insert(0, "/opt/trn_rl_repo")

import numpy as np

GAMMA = 0.99
LAM = 0.95
C_COEF = GAMMA * LAM

B, S = 2048, 8192
N_CORES = 8
ROWS = B // N_CORES  # 256 rows per core
P = 128  # SBUF partitions
N_TILES = ROWS // P  # 2 row-tiles per core
CHUNK = 2048
N_CHUNKS = S // CHUNK

_CACHE: dict = {}


def _build():
    import concourse.bacc as bacc
    import concourse.mybir as mybir
    from concourse.tile import TileContext

    f32 = mybir.dt.float32
    add = mybir.AluOpType.add
    sub = mybir.AluOpType.subtract
    mult = mybir.AluOpType.mult

    nc = bacc.Bacc("TRN2", target_bir_lowering=False, name="gae8")
    r = nc.dram_tensor("rewards", [ROWS, S], f32, kind="ExternalInput")
    v = nc.dram_tensor("values", [ROWS, S], f32, kind="ExternalInput")
    nv = nc.dram_tensor("next_values", [ROWS], f32, kind="ExternalInput")
    adv = nc.dram_tensor("adv", [ROWS, S], f32, kind="ExternalOutput")
    ret = nc.dram_tensor("ret", [ROWS, S], f32, kind="ExternalOutput")

    # Returns satisfy their own backward recurrence, which needs one fewer
    # elementwise pass than the advantages form:
    #   B_t = e_t + c*B_{t+1},  e_t = r_t + gamma*(1-lam)*v_{t+1},  B_S = nv
    #   returns = B, advantages = B - v
    g1ml = GAMMA * (1.0 - LAM)

    with TileContext(nc) as tc:
        with (
            tc.tile_pool(name="cpool", bufs=1) as cpool,
            tc.tile_pool(name="pool", bufs=4) as pool,
        ):
            c_t = cpool.tile([P, 1], f32)
            nc.vector.memset(c_t[:, :], C_COEF)

            for t in range(N_TILES):
                rows = slice(t * P, (t + 1) * P)
                prev_ret = None
                # right-to-left so the scan carry chains across chunks
                for k in range(N_CHUNKS - 1, -1, -1):
                    col = k * CHUNK
                    v_t = pool.tile([P, CHUNK + 1], f32)
                    r_t = pool.tile([P, CHUNK], f32)
                    ret_t = pool.tile([P, CHUNK], f32)
                    adv_t = pool.tile([P, CHUNK], f32)

                    # v_t holds CHUNK+1 columns: v[:, col:col+CHUNK] plus the
                    # successor column (next chunk's first value, or
                    # next_values at the right edge).
                    if k == N_CHUNKS - 1:
                        nc.sync.dma_start(
                            out=v_t[:, 0:CHUNK], in_=v[rows, col : col + CHUNK]
                        )
                        nc.sync.dma_start(
                            out=v_t[:, CHUNK : CHUNK + 1],
                            in_=nv[t * P : (t + 1) * P].unsqueeze(1),
                        )
                    else:
                        nc.sync.dma_start(
                            out=v_t[:, :], in_=v[rows, col : col + CHUNK + 1]
                        )
                    nc.sync.dma_start(out=r_t[:, :], in_=r[rows, col : col + CHUNK])

                    # e = g1ml * v_next + r  (in place over r_t)
                    nc.vector.scalar_tensor_tensor(
                        out=r_t[:, :],
                        in0=v_t[:, 1 : CHUNK + 1],
                        scalar=g1ml,
                        in1=r_t[:, :],
                        op0=mult,
                        op1=add,
                    )
                    # backward recurrence over reversed views:
                    # state = c*state + e -> returns; carry chains via initial.
                    # Rightmost chunk: initial = next_values (v_t's extra col).
                    init = (
                        v_t[:, CHUNK : CHUNK + 1]
                        if prev_ret is None
                        else prev_ret[:, 0:1]
                    )
                    nc.vector.tensor_tensor_scan(
                        out=ret_t[:, ::-1],
                        data0=c_t[:, :].broadcast_to([P, CHUNK]),
                        data1=r_t[:, ::-1],
                        initial=init,
                        op0=mult,
                        op1=add,
                    )
                    # advantages = returns - v (DVE: GpSimd contends for SBUF
                    # ports with DVE and halves scan throughput)
                    nc.vector.tensor_tensor(
                        out=adv_t[:, :],
                        in0=ret_t[:, :],
                        in1=v_t[:, 0:CHUNK],
                        op=sub,
                    )

                    # stores go out the scalar-engine HWDGE ring
                    # (qActDynamicHW) so they don't FIFO-serialize behind
                    # later chunks' loads on the sync ring (qSPDynamicHW).
                    nc.scalar.dma_start(
                        out=ret[rows, col : col + CHUNK], in_=ret_t[:, :]
                    )
                    nc.scalar.dma_start(
                        out=adv[rows, col : col + CHUNK], in_=adv_t[:, :]
                    )
                    prev_ret = ret_t
    nc.finalize()
    return nc


def _get_nc():
    if "nc" not in _CACHE:
        _CACHE["nc"] = _build()
    return _CACHE["nc"]


def _run(rewards, values, next_values, **spmd_kwargs):
    """Shard over cores, run the Bass kernel, return BassKernelResults."""
    from concourse.bass_utils import run_bass_kernel_spmd

    nc = _get_nc()
    in_maps = []
    for c in range(N_CORES):
        sl = slice(c * ROWS, (c + 1) * ROWS)
        in_maps.append(
            {
                "rewards": np.ascontiguousarray(rewards[sl], dtype=np.float32),
                "values": np.ascontiguousarray(values[sl], dtype=np.float32),
                "next_values": np.ascontiguousarray(
                    next_values[sl], dtype=np.float32
                ),
            }
        )
    return run_bass_kernel_spmd(
        nc, in_maps, core_ids=list(range(N_CORES)), **spmd_kwargs
    )


def kernel(rewards, values, next_values):
    res = _run(rewards, values, next_values)
    advantages = np.concatenate([res.results[c]["adv"] for c in range(N_CORES)], 0)
    returns = np.concatenate([res.results[c]["ret"] for c in range(N_CORES)], 0)
    return advantages, returns
